# revision 7
# baseline (speedup 1.0000x reference)
"""Trainium2 Bass kernel for nn_NodeRouter (MoE routing).

Reference computation (B=8, L=2048, D=1024, NUM_NODES=64, NUM_HEADS=8, K=2):
    v = x @ W.T + b                      # (B, L, 512) gate logits
    val, ind = top_k(v, 2)
    item = val * x * gate_values[ind]    # (B, L, 2, D)
    out[b, n] = sum of item over (l, k) routed to node n    # (B, 64, D)
    counts[b, n] = number of (l, k) entries routed to node n

Sharding: data-parallel over batch B -> 8 cores, one batch element per core.
W / b / gate_values replicated (host pre-transposes layouts).

Per-core algorithm (no explicit top-k indices, no scatter):
    phase 1: v = x @ W.T + b via PE (l-tiles of 128 tokens x 512 slots).
             DVE `max` op gives top-8 per token -> threshold t = 2nd max.
             C[l, s] = v[l,s] if v[l,s] >= t_l else 0   (routing matrix)
             macc[l, s] += (v >= t)                      (count accumulator)
    phase 2: A^T[d, s] = sum_l x[l, d] * C[l, s]  via PE (x natural as lhsT)
    epilog:  outT[d, n] = sum_h A^T[d, n*8+h] * gateT[d, n*8+h]
             counts[n] = sum_l sum_h (v >= t)[l, n*8+h]  via ones-matmul
Host gathers: out[b] = outT.T, counts int32.
"""

import os
import sys
sys.path.insert(0, "/opt/trn_rl_repo")

import numpy as np

import concourse.bacc as bacc
import concourse.bass as bass
import concourse.mybir as mybir
import concourse.tile as tile
from concourse.bass_utils import run_bass_kernel_spmd

B, L, D = 8, 2048, 1024
NN, NH, K = 64, 8, 2
S = NN * NH          # 512 gate slots
NT = L // 128        # 16 token tiles
DC = D // 128        # 8 contraction chunks
F32 = mybir.dt.float32

# matmul input dtype: float32r runs 4x faster on the PE than float32
# (1 cycle/row vs 4) at slightly relaxed precision. Selection exactness is
# verified in test.py against the fixed problem seed.
MM_DT = {
    "f32": mybir.dt.float32,
    "f32r": mybir.dt.float32r,
}[os.environ.get("NODEROUTER_MM_DT", "f32r")]


def _mm(ap):
    """Bitcast an f32 AP to the matmul input dtype."""
    if MM_DT == F32:
        return ap
    return ap.bitcast(MM_DT)


def build_nc():
    nc = bacc.Bacc(
        "TRN2",
        target_bir_lowering=False,
        debug=False,
        num_devices=B,
    )

    xT = nc.dram_tensor("xT", [D, L], F32, kind="ExternalInput").ap()
    xn = nc.dram_tensor("xn", [L, D], F32, kind="ExternalInput").ap()
    wT = nc.dram_tensor("wT", [D, S], F32, kind="ExternalInput").ap()
    bb = nc.dram_tensor("bb", [128, S], F32, kind="ExternalInput").ap()
    gT = nc.dram_tensor("gT", [D, S], F32, kind="ExternalInput").ap()
    outT = nc.dram_tensor("outT", [D, NN], F32, kind="ExternalOutput").ap()
    cnt = nc.dram_tensor("cnt", [1, NN], F32, kind="ExternalOutput").ap()

    ts = bass.ts
    ige = mybir.AluOpType.is_ge
    mul = mybir.AluOpType.mult
    add = mybir.AluOpType.add

    with tile.TileContext(nc) as tc, tc.tile_pool(name="const", bufs=1) as const:

        # Resident SBUF tensors
        wt_sb = const.tile([128, DC * S], F32)        # W^T, chunk c at cols [c*S, (c+1)*S)
        for c in range(DC):
            nc.sync.dma_start(wt_sb[:, ts(c, S)], wT[c * 128:(c + 1) * 128, :])
        bb_sb = const.tile([128, S], F32)
        nc.sync.dma_start(bb_sb[:], bb[:])
        xt_sb = const.tile([128, DC * L], F32)        # x^T, chunk c at cols [c*L, (c+1)*L)
        for c in range(DC):
            nc.sync.dma_start(xt_sb[:, ts(c, L)], xT[c * 128:(c + 1) * 128, :])
        gt_sb = const.tile([128, DC * S], F32)
        for c in range(DC):
            nc.sync.dma_start(gt_sb[:, ts(c, S)], gT[c * 128:(c + 1) * 128, :])
        c_all = const.tile([128, NT * S], F32)        # routing matrix, tile i at cols [i*S, ...)
        macc = const.tile([128, S], F32)              # mask accumulator
        nc.vector.memset(macc[:], 0.0)
        ones_sb = const.tile([128, 1], F32)
        nc.vector.memset(ones_sb[:], 1.0)
        cnt_sb = const.tile([1, NN], F32)
        outT_sb = const.tile([128, DC * NN], F32)

        # ---- Phase 1: logits, top-2 threshold, routing matrix C ----
        with tc.tile_pool(name="vps", bufs=2, space="PSUM") as vps_pool, \
             tc.tile_pool(name="cntps", bufs=1, space="PSUM") as cnt_pool, \
             tc.tile_pool(name="work", bufs=2) as work_pool, \
             tc.tile_pool(name="mx", bufs=2) as mx_pool:
            for i in range(NT):
                v_ps = vps_pool.tile([128, S], F32)
                for c in range(DC):
                    nc.tensor.matmul(
                        v_ps[:],
                        lhsT=_mm(xt_sb[:, c * L + i * 128: c * L + (i + 1) * 128]),
                        rhs=_mm(wt_sb[:, ts(c, S)]),
                        start=(c == 0),
                        stop=(c == DC - 1),
                    )
                vb = work_pool.tile([128, S], F32)
                nc.vector.tensor_add(vb[:], v_ps[:], bb_sb[:])
                mx = mx_pool.tile([128, 8], F32)
                nc.vector.max(mx[:], vb[:])
                t_ap = mx[:, K - 1:K]  # 2nd-largest per token
                # C = (vb >= t) * vb ; macc += (vb >= t)
                nc.vector.scalar_tensor_tensor(
                    c_all[:, ts(i, S)], vb[:], t_ap, vb[:], op0=ige, op1=mul
                )
                nc.vector.scalar_tensor_tensor(
                    macc[:], vb[:], t_ap, macc[:], op0=ige, op1=add
                )

            # counts: reduce heads on free dim, then sum over partitions via matmul
            cnt_h = work_pool.tile([128, NN], F32)
            nc.vector.tensor_reduce(
                cnt_h[:],
                macc[:].rearrange("p (n h) -> p n h", h=NH),
                axis=mybir.AxisListType.X,
                op=add,
            )
            cnt_ps = cnt_pool.tile([1, NN], F32)
            nc.tensor.matmul(
                cnt_ps[:], lhsT=_mm(ones_sb[:]), rhs=_mm(cnt_h[:]),
                start=True, stop=True,
            )
            nc.vector.tensor_copy(cnt_sb[:], cnt_ps[:])

        # ---- Phase 2: A^T[d, s] = sum_l x[l, d] * C[l, s] ----
        with tc.tile_pool(name="atps", bufs=8, space="PSUM") as at_pool, \
             tc.tile_pool(name="xn", bufs=3) as xn_pool, \
             tc.tile_pool(name="gw", bufs=2) as gw_pool:
            at_ps = [
                at_pool.tile([128, S], F32, name=f"at{c}", tag=f"at{c}", bufs=1)
                for c in range(DC)
            ]
            for i in range(NT):
                xn_sb = xn_pool.tile([128, D], F32)
                nc.sync.dma_start(xn_sb[:], xn[i * 128:(i + 1) * 128, :])
                for c in range(DC):
                    nc.tensor.matmul(
                        at_ps[c][:],
                        lhsT=_mm(xn_sb[:, ts(c, 128)]),
                        rhs=_mm(c_all[:, ts(i, S)]),
                        start=(i == 0),
                        stop=(i == NT - 1),
                    )

            # ---- Epilogue: gate multiply + head reduction ----
            for c in range(DC):
                gtile = gw_pool.tile([128, S], F32)
                nc.vector.tensor_mul(gtile[:], at_ps[c][:], gt_sb[:, ts(c, S)])
                nc.vector.tensor_reduce(
                    outT_sb[:, ts(c, NN)],
                    gtile[:].rearrange("p (n h) -> p n h", h=NH),
                    axis=mybir.AxisListType.X,
                    op=add,
                )
                nc.sync.dma_start(
                    outT[c * 128:(c + 1) * 128, :], outT_sb[:, ts(c, NN)]
                )
            nc.sync.dma_start(cnt[:], cnt_sb[:])

    nc.compile()
    return nc


_NC_CACHE = None


def _get_nc():
    global _NC_CACHE
    if _NC_CACHE is None:
        _NC_CACHE = build_nc()
    return _NC_CACHE


def kernel(x, W, b, gate_values, _trace=False):
    x = np.asarray(x, dtype=np.float32)
    W = np.asarray(W, dtype=np.float32)
    b = np.asarray(b, dtype=np.float32)
    gate_values = np.asarray(gate_values, dtype=np.float32)

    # Replicated, layout-prepped weights
    wT_np = np.ascontiguousarray(W.T)                                # (D, S)
    bb_np = np.ascontiguousarray(np.broadcast_to(b, (128, S)))       # (128, S)
    gT_np = np.ascontiguousarray(gate_values.reshape(S, D).T)        # (D, S)

    in_maps = []
    for i in range(B):
        xb = np.ascontiguousarray(x[i])
        in_maps.append({
            "xT": np.ascontiguousarray(xb.T),
            "xn": xb,
            "wT": wT_np,
            "bb": bb_np,
            "gT": gT_np,
        })

    nc = _get_nc()
    res = run_bass_kernel_spmd(nc, in_maps, core_ids=list(range(B)), trace=_trace)

    out = np.empty((B, NN, D), dtype=np.float32)
    counts = np.empty((B, NN), dtype=np.int32)
    for i in range(B):
        out[i] = res.results[i]["outT"].T
        counts[i] = np.rint(res.results[i]["cnt"][0]).astype(np.int32)

    if _trace:
        return (out, counts), res
    return out, counts


if __name__ == "__main__":
    rng = np.random.default_rng(0)
    inputs = {
        "x": rng.standard_normal((B, L, D), dtype=np.float32),
        "W": (rng.standard_normal((S, D)) * 0.02).astype(np.float32),
        "b": (rng.standard_normal((S,)) * 0.01).astype(np.float32),
        "gate_values": rng.standard_normal((NN, NH, D)).astype(np.float32),
    }
    out, counts = kernel(**inputs)
    print("out", out.shape, out.dtype, "counts", counts.shape, counts.dtype)
    print("counts sum per batch:", counts.sum(axis=1))


# revision 10
# speedup vs baseline: 1.3739x; 1.3739x over previous
"""Trainium2 Bass kernel for nn_NodeRouter (MoE routing).

Reference computation (B=8, L=2048, D=1024, NUM_NODES=64, NUM_HEADS=8, K=2):
    v = x @ W.T + b                      # (B, L, 512) gate logits
    val, ind = top_k(v, 2)
    item = val * x * gate_values[ind]    # (B, L, 2, D)
    out[b, n] = sum of item over (l, k) routed to node n    # (B, 64, D)
    counts[b, n] = number of (l, k) entries routed to node n

Sharding: data-parallel over batch B -> 8 cores, one batch element per core.
W / b / gate_values replicated (host pre-transposes layouts).

Per-core algorithm (no explicit top-k indices, no scatter):
    phase 1: v = x @ W.T + b via PE (l-tiles of 128 tokens x 512 slots).
             DVE `max` op gives top-8 per token -> threshold t = 2nd max.
             C[l, s] = v[l,s] if v[l,s] >= t_l else 0   (routing matrix)
             macc[l, s] += (v >= t)                      (count accumulator)
    phase 2: A^T[d, s] = sum_l x[l, d] * C[l, s]  via PE (x natural as lhsT)
    epilog:  outT[d, n] = sum_h A^T[d, n*8+h] * gateT[d, n*8+h]
             counts[n] = sum_l sum_h (v >= t)[l, n*8+h]  via ones-matmul
Host gathers: out[b] = outT.T, counts int32.
"""

import os
import sys
sys.path.insert(0, "/opt/trn_rl_repo")

import numpy as np

import concourse.bacc as bacc
import concourse.bass as bass
import concourse.mybir as mybir
import concourse.tile as tile
from concourse.bass_utils import run_bass_kernel_spmd

B, L, D = 8, 2048, 1024
NN, NH, K = 64, 8, 2
S = NN * NH          # 512 gate slots
NT = L // 128        # 16 token tiles
DC = D // 128        # 8 contraction chunks
F32 = mybir.dt.float32

# matmul input dtype: float32r runs 4x faster on the PE than float32
# (1 cycle/row vs 4) at slightly relaxed precision. Selection exactness is
# verified in test.py against the fixed problem seed.
MM_DT = {
    "f32": mybir.dt.float32,
    "f32r": mybir.dt.float32r,
}[os.environ.get("NODEROUTER_MM_DT", "f32r")]




def build_nc():
    nc = bacc.Bacc(
        "TRN2",
        target_bir_lowering=False,
        debug=False,
        num_devices=B,
    )

    xT = nc.dram_tensor("xT", [D, L], MM_DT, kind="ExternalInput").ap()
    xn = nc.dram_tensor("xn", [L, D], MM_DT, kind="ExternalInput").ap()
    wT = nc.dram_tensor("wT", [D, S], MM_DT, kind="ExternalInput").ap()
    bb = nc.dram_tensor("bb", [128, S], F32, kind="ExternalInput").ap()
    gT = nc.dram_tensor("gT", [D, S], F32, kind="ExternalInput").ap()
    outT = nc.dram_tensor("outT", [D, NN], F32, kind="ExternalOutput").ap()
    cnt = nc.dram_tensor("cnt", [1, NN], F32, kind="ExternalOutput").ap()

    ts = bass.ts
    ige = mybir.AluOpType.is_ge
    mul = mybir.AluOpType.mult
    add = mybir.AluOpType.add

    with tile.TileContext(nc) as tc, tc.tile_pool(name="const", bufs=1) as const:

        # Resident SBUF tensors
        wt_sb = const.tile([128, DC * S], MM_DT)        # W^T, chunk c at cols [c*S, (c+1)*S)
        for c in range(DC):
            nc.sync.dma_start(wt_sb[:, ts(c, S)], wT[c * 128:(c + 1) * 128, :])
        bb_sb = const.tile([128, S], F32)
        nc.sync.dma_start(bb_sb[:], bb[:])
        xt_sb = const.tile([128, DC * L], MM_DT)        # x^T, chunk c at cols [c*L, (c+1)*L)
        for c in range(DC):
            nc.sync.dma_start(xt_sb[:, ts(c, L)], xT[c * 128:(c + 1) * 128, :])
        gt_sb = const.tile([128, DC * S], F32)
        for c in range(DC):
            nc.sync.dma_start(gt_sb[:, ts(c, S)], gT[c * 128:(c + 1) * 128, :])
        c_all = const.tile([128, NT * S], MM_DT)        # routing matrix, tile i at cols [i*S, ...)
        macc = const.tile([128, S], F32)              # mask accumulator
        nc.vector.memset(macc[:], 0.0)
        ones_sb = const.tile([128, 1], MM_DT)
        nc.vector.tensor_scalar(
            ones_sb[:], bb_sb[:, 0:1], 0.0, 1.0,
            op0=mul, op1=add,
        )
        cnt_sb = const.tile([1, NN], F32)
        outT_sb = const.tile([128, DC * NN], F32)

        # ---- Phase 1: logits, top-2 threshold, routing matrix C ----
        with tc.tile_pool(name="vps", bufs=2, space="PSUM") as vps_pool, \
             tc.tile_pool(name="cntps", bufs=1, space="PSUM") as cnt_pool, \
             tc.tile_pool(name="work", bufs=2) as work_pool, \
             tc.tile_pool(name="mx", bufs=2) as mx_pool:
            for i in range(NT):
                v_ps = vps_pool.tile([128, S], F32)
                for c in range(DC):
                    nc.tensor.matmul(
                        v_ps[:],
                        lhsT=xt_sb[:, c * L + i * 128: c * L + (i + 1) * 128],
                        rhs=wt_sb[:, ts(c, S)],
                        start=(c == 0),
                        stop=(c == DC - 1),
                    )
                vb = work_pool.tile([128, S], F32)
                nc.vector.tensor_add(vb[:], v_ps[:], bb_sb[:])
                mx = mx_pool.tile([128, 8], F32)
                nc.vector.max(mx[:], vb[:])
                t_ap = mx[:, K - 1:K]  # 2nd-largest per token
                # C = (vb >= t) * vb ; macc += (vb >= t)
                nc.vector.scalar_tensor_tensor(
                    c_all[:, ts(i, S)], vb[:], t_ap, vb[:], op0=ige, op1=mul
                )
                nc.vector.scalar_tensor_tensor(
                    macc[:], vb[:], t_ap, macc[:], op0=ige, op1=add
                )

            # counts: reduce heads on free dim, then sum over partitions via matmul
            cnt_h = work_pool.tile([128, NN], MM_DT)
            with nc.allow_low_precision(reason="counts are small exact ints"):
                nc.vector.tensor_reduce(
                    cnt_h[:],
                    macc[:].rearrange("p (n h) -> p n h", h=NH),
                    axis=mybir.AxisListType.X,
                    op=add,
                )
            cnt_ps = cnt_pool.tile([1, NN], F32)
            nc.tensor.matmul(
                cnt_ps[:], lhsT=ones_sb[:], rhs=cnt_h[:],
                start=True, stop=True,
            )
            nc.vector.tensor_copy(cnt_sb[:], cnt_ps[:])

        # ---- Phase 2: A^T[d, s] = sum_l x[l, d] * C[l, s] ----
        with tc.tile_pool(name="atps", bufs=8, space="PSUM") as at_pool, \
             tc.tile_pool(name="xn", bufs=3) as xn_pool, \
             tc.tile_pool(name="gw", bufs=2) as gw_pool:
            at_ps = [
                at_pool.tile([128, S], F32, name=f"at{c}", tag=f"at{c}", bufs=1)
                for c in range(DC)
            ]
            for i in range(NT):
                xn_sb = xn_pool.tile([128, D], MM_DT)
                nc.sync.dma_start(xn_sb[:], xn[i * 128:(i + 1) * 128, :])
                for c in range(DC):
                    nc.tensor.matmul(
                        at_ps[c][:],
                        lhsT=xn_sb[:, ts(c, 128)],
                        rhs=c_all[:, ts(i, S)],
                        start=(i == 0),
                        stop=(i == NT - 1),
                    )

            # ---- Epilogue: gate multiply + head reduction ----
            for c in range(DC):
                gtile = gw_pool.tile([128, S], F32)
                nc.vector.tensor_mul(gtile[:], at_ps[c][:], gt_sb[:, ts(c, S)])
                nc.vector.tensor_reduce(
                    outT_sb[:, ts(c, NN)],
                    gtile[:].rearrange("p (n h) -> p n h", h=NH),
                    axis=mybir.AxisListType.X,
                    op=add,
                )
                nc.sync.dma_start(
                    outT[c * 128:(c + 1) * 128, :], outT_sb[:, ts(c, NN)]
                )
            nc.sync.dma_start(cnt[:], cnt_sb[:])

    nc.compile()
    return nc


_NC_CACHE = None


def _get_nc():
    global _NC_CACHE
    if _NC_CACHE is None:
        _NC_CACHE = build_nc()
    return _NC_CACHE


def kernel(x, W, b, gate_values, _trace=False):
    x = np.asarray(x, dtype=np.float32)
    W = np.asarray(W, dtype=np.float32)
    b = np.asarray(b, dtype=np.float32)
    gate_values = np.asarray(gate_values, dtype=np.float32)

    # Replicated, layout-prepped weights
    wT_np = np.ascontiguousarray(W.T)                                # (D, S)
    bb_np = np.ascontiguousarray(np.broadcast_to(b, (128, S)))       # (128, S)
    gT_np = np.ascontiguousarray(gate_values.reshape(S, D).T)        # (D, S)

    in_maps = []
    for i in range(B):
        xb = np.ascontiguousarray(x[i])
        in_maps.append({
            "xT": np.ascontiguousarray(xb.T),
            "xn": xb,
            "wT": wT_np,
            "bb": bb_np,
            "gT": gT_np,
        })

    nc = _get_nc()
    res = run_bass_kernel_spmd(nc, in_maps, core_ids=list(range(B)), trace=_trace)

    out = np.empty((B, NN, D), dtype=np.float32)
    counts = np.empty((B, NN), dtype=np.int32)
    for i in range(B):
        out[i] = res.results[i]["outT"].T
        counts[i] = np.rint(res.results[i]["cnt"][0]).astype(np.int32)

    if _trace:
        return (out, counts), res
    return out, counts


if __name__ == "__main__":
    rng = np.random.default_rng(0)
    inputs = {
        "x": rng.standard_normal((B, L, D), dtype=np.float32),
        "W": (rng.standard_normal((S, D)) * 0.02).astype(np.float32),
        "b": (rng.standard_normal((S,)) * 0.01).astype(np.float32),
        "gate_values": rng.standard_normal((NN, NH, D)).astype(np.float32),
    }
    out, counts = kernel(**inputs)
    print("out", out.shape, out.dtype, "counts", counts.shape, counts.dtype)
    print("counts sum per batch:", counts.sum(axis=1))


# revision 11
# speedup vs baseline: 1.6926x; 1.2320x over previous
"""Trainium2 Bass kernel for nn_NodeRouter (MoE routing) — v4.

v4: phase-1 logits computed with a 3-term compensated split so top-2
selection is fp32-exact at ~3x the f32r matmul rate:
    v = xh*Wh (f32r) + xl*W (bf16) + x*Wl (bf16),  error ~ xl*Wl ~ 2^-24
where xh = round_m(x), xl = bf16(x - xh), Wh = round_m(W), Wl = bf16(W - Wh).

v3 changes over v2 (both-matmuls f32r):
  - xT shipped in tile-major layout (xTt): one contiguous 512KB DMA per
    l-tile, so phase 1 starts after ~2.5MB instead of ~12MB.
  - bias folded into the logits matmul as a K=1 accumulation step
    (kills the DVE bias-add pass; v stays in PSUM, max/stt read PSUM).
  - count accumulation moved to GpSimd, reading C (SBUF) via (C != 0).
  - phase-2 dtype selectable (f32r or bf16) via NODEROUTER_P2_DT.
"""

import os
import sys
sys.path.insert(0, "/opt/trn_rl_repo")

import ml_dtypes
import numpy as np

import concourse.bacc as bacc
import concourse.bass as bass
import concourse.mybir as mybir
import concourse.tile as tile
from concourse.bass_utils import run_bass_kernel_spmd

B, L, D = 8, 2048, 1024
NN, NH, K = 64, 8, 2
S = NN * NH          # 512 gate slots
NT = L // 128        # 16 token tiles
DC = D // 128        # 8 contraction chunks
F32 = mybir.dt.float32

MM_DT = {
    "f32": mybir.dt.float32,
    "f32r": mybir.dt.float32r,
}[os.environ.get("NODEROUTER_MM_DT", "f32r")]
P2_DT = {
    "f32": mybir.dt.float32,
    "f32r": mybir.dt.float32r,
    "bf16": mybir.dt.bfloat16,
}[os.environ.get("NODEROUTER_P2_DT", "f32r")]


def build_nc():
    nc = bacc.Bacc(
        "TRN2",
        target_bir_lowering=False,
        debug=False,
        num_devices=B,
    )

    BF16 = mybir.dt.bfloat16
    F32R = mybir.dt.float32r
    # tile-major x^T: row block i holds (128 d, 8*128) where cols [c*128, ...)
    # are chunk c of l-tile i, partitions = d within chunk, cols = token l.
    xTh = nc.dram_tensor("xTh", [NT * 128, DC * 128], F32R, kind="ExternalInput").ap()
    xTl = nc.dram_tensor("xTl", [NT * 128, DC * 128], BF16, kind="ExternalInput").ap()
    xTb = nc.dram_tensor("xTb", [NT * 128, DC * 128], BF16, kind="ExternalInput").ap()
    xn = nc.dram_tensor("xn", [L, D], P2_DT, kind="ExternalInput").ap()
    wTh = nc.dram_tensor("wTh", [D, S], F32R, kind="ExternalInput").ap()
    wTb = nc.dram_tensor("wTb", [D, S], BF16, kind="ExternalInput").ap()
    wTl = nc.dram_tensor("wTl", [D, S], BF16, kind="ExternalInput").ap()
    br = nc.dram_tensor("br", [1, S], F32R, kind="ExternalInput").ap()
    gT = nc.dram_tensor("gT", [D, S], F32, kind="ExternalInput").ap()
    outT = nc.dram_tensor("outT", [D, NN], F32, kind="ExternalOutput").ap()
    cnt = nc.dram_tensor("cnt", [1, NN], F32, kind="ExternalOutput").ap()

    ts = bass.ts
    ige = mybir.AluOpType.is_ge
    ine = mybir.AluOpType.not_equal
    mul = mybir.AluOpType.mult
    add = mybir.AluOpType.add

    with tile.TileContext(nc) as tc, tc.tile_pool(name="const", bufs=1) as const:

        # Resident SBUF tensors
        wth_sb = const.tile([128, DC * S], F32R)      # W^T hi, chunk c at cols [c*S, ...)
        wtb_sb = const.tile([128, DC * S], BF16)      # W^T bf16
        wtl_sb = const.tile([128, DC * S], BF16)      # W^T lo residual
        for c in range(DC):
            nc.sync.dma_start(wth_sb[:, ts(c, S)], wTh[c * 128:(c + 1) * 128, :])
            nc.sync.dma_start(wtb_sb[:, ts(c, S)], wTb[c * 128:(c + 1) * 128, :])
            nc.sync.dma_start(wtl_sb[:, ts(c, S)], wTl[c * 128:(c + 1) * 128, :])
        br_sb = const.tile([1, S], F32R)
        nc.sync.dma_start(br_sb[:], br[:])
        gt_sb = const.tile([128, DC * S], F32)
        for c in range(DC):
            nc.sync.dma_start(gt_sb[:, ts(c, S)], gT[c * 128:(c + 1) * 128, :])
        c_all = const.tile([128, NT * S], P2_DT)      # routing matrix, tile i at cols [i*S, ...)
        macc = const.tile([128, S], F32)              # mask accumulator
        nc.vector.memset(macc[:], 0.0)
        # ones vectors built from real data (x*0 + 1), since memset can't
        # write f32r
        ones1p = const.tile([1, 128], F32R)           # K=1 bias-matmul lhsT
        nc.vector.tensor_scalar(ones1p[:], br_sb[:, 0:128].bitcast(F32), 0.0, 1.0, op0=mul, op1=add)
        ones_sb = const.tile([128, 1], F32R)          # counts-matmul lhsT
        nc.vector.tensor_scalar(ones_sb[:], gt_sb[:, 0:1], 0.0, 1.0, op0=mul, op1=add)
        cnt_sb = const.tile([1, NN], F32)
        outT_sb = const.tile([128, DC * NN], F32)

        # ---- Phase 1: logits (+bias), top-2 threshold, routing matrix C ----
        with tc.tile_pool(name="vps", bufs=2, space="PSUM") as vps_pool, \
             tc.tile_pool(name="cntps", bufs=1, space="PSUM") as cnt_pool, \
             tc.tile_pool(name="xtt", bufs=3) as xtt_pool, \
             tc.tile_pool(name="work", bufs=2) as work_pool, \
             tc.tile_pool(name="mx", bufs=2) as mx_pool:
            for i in range(NT):
                xh_sb = xtt_pool.tile([128, DC * 128], F32R, name="xh_sb", tag="xh")
                nc.sync.dma_start(xh_sb[:], xTh[i * 128:(i + 1) * 128, :])
                xl_sb = xtt_pool.tile([128, DC * 128], BF16, name="xl_sb", tag="xl")
                nc.sync.dma_start(xl_sb[:], xTl[i * 128:(i + 1) * 128, :])
                xb_sb = xtt_pool.tile([128, DC * 128], BF16, name="xb_sb", tag="xb")
                nc.sync.dma_start(xb_sb[:], xTb[i * 128:(i + 1) * 128, :])
                v_ps = vps_pool.tile([128, S], F32)
                # bias via K=1 broadcast matmul, then 3 compensated terms
                # per contraction chunk: xh*Wh (f32r) + xl*W (bf16) + x*Wl (bf16)
                nc.tensor.matmul(
                    v_ps[:], lhsT=ones1p[:], rhs=br_sb[:],
                    start=True, stop=False,
                )
                for c in range(DC):
                    nc.tensor.matmul(
                        v_ps[:], lhsT=xh_sb[:, ts(c, 128)], rhs=wth_sb[:, ts(c, S)],
                        start=False, stop=False,
                    )
                    nc.tensor.matmul(
                        v_ps[:], lhsT=xl_sb[:, ts(c, 128)], rhs=wtb_sb[:, ts(c, S)],
                        start=False, stop=False,
                    )
                    nc.tensor.matmul(
                        v_ps[:], lhsT=xb_sb[:, ts(c, 128)], rhs=wtl_sb[:, ts(c, S)],
                        start=False, stop=(c == DC - 1),
                    )
                # ACT engine moves v to SBUF (frees DVE from the copy)
                vb = work_pool.tile([128, S], F32)
                nc.scalar.copy(vb[:], v_ps[:])
                mx = mx_pool.tile([128, 8], F32)
                nc.vector.max(mx[:], vb[:])
                t_ap = mx[:, K - 1:K]  # 2nd-largest per token
                # C = (v >= t) * v
                nc.vector.scalar_tensor_tensor(
                    c_all[:, ts(i, S)], vb[:], t_ap, vb[:], op0=ige, op1=mul
                )
                # macc += (v >= t)
                nc.vector.scalar_tensor_tensor(
                    macc[:], vb[:], t_ap, macc[:],
                    op0=ige, op1=add,
                )

            # counts: reduce heads on free dim, then sum over partitions via matmul
            cnt_h = work_pool.tile([128, NN], F32R)
            with nc.allow_low_precision(reason="counts are small exact ints"):
                nc.vector.tensor_reduce(
                    cnt_h[:],
                    macc[:].rearrange("p (n h) -> p n h", h=NH),
                    axis=mybir.AxisListType.X,
                    op=add,
                )
            cnt_ps = cnt_pool.tile([1, NN], F32)
            nc.tensor.matmul(
                cnt_ps[:], lhsT=ones_sb[:], rhs=cnt_h[:],
                start=True, stop=True,
            )
            nc.vector.tensor_copy(cnt_sb[:], cnt_ps[:])

        # ---- Phase 2: A^T[d, s] = sum_l x[l, d] * C[l, s] ----
        with tc.tile_pool(name="atps", bufs=8, space="PSUM") as at_pool, \
             tc.tile_pool(name="xnp", bufs=3) as xn_pool, \
             tc.tile_pool(name="gw", bufs=2) as gw_pool:
            at_ps = [
                at_pool.tile([128, S], F32, name=f"at{c}", tag=f"at{c}", bufs=1)
                for c in range(DC)
            ]
            for i in range(NT):
                xn_sb = xn_pool.tile([128, D], P2_DT)
                nc.sync.dma_start(xn_sb[:], xn[i * 128:(i + 1) * 128, :])
                for c in range(DC):
                    nc.tensor.matmul(
                        at_ps[c][:],
                        lhsT=xn_sb[:, ts(c, 128)],
                        rhs=c_all[:, ts(i, S)],
                        start=(i == 0),
                        stop=(i == NT - 1),
                    )

            # ---- Epilogue: gate multiply + head reduction ----
            for c in range(DC):
                gtile = gw_pool.tile([128, S], F32)
                nc.vector.tensor_mul(gtile[:], at_ps[c][:], gt_sb[:, ts(c, S)])
                nc.vector.tensor_reduce(
                    outT_sb[:, ts(c, NN)],
                    gtile[:].rearrange("p (n h) -> p n h", h=NH),
                    axis=mybir.AxisListType.X,
                    op=add,
                )
                nc.sync.dma_start(
                    outT[c * 128:(c + 1) * 128, :], outT_sb[:, ts(c, NN)]
                )
            nc.sync.dma_start(cnt[:], cnt_sb[:])

    nc.compile()
    return nc


_NC_CACHE = None


def _get_nc():
    global _NC_CACHE
    if _NC_CACHE is None:
        _NC_CACHE = build_nc()
    return _NC_CACHE


def _p2_np(a):
    if P2_DT == mybir.dt.bfloat16:
        return a.astype(ml_dtypes.bfloat16)
    return a


# f32r operand mantissa width (explicit bits) used for the hi/lo split.
SPLIT_MANT = int(os.environ.get("NODEROUTER_SPLIT_MANT", "11"))


def _round_mant(a, mant=SPLIT_MANT):
    """Round-to-nearest keeping `mant` explicit mantissa bits (f32-exact)."""
    am = np.abs(a)
    e = np.floor(np.log2(am, where=am > 0, out=np.zeros_like(a)))
    scale = np.float32(2.0) ** (e - mant).astype(np.float32)
    return (np.round(a / scale) * scale).astype(np.float32)


def _tile_major(xb):
    """[i*128+l, c*128+p] -> [i, p, c, l] tile-major transpose, kept 2D."""
    return np.ascontiguousarray(
        xb.reshape(NT, 128, DC, 128).transpose(0, 3, 2, 1).reshape(NT * 128, D)
    )


def kernel(x, W, b, gate_values, _trace=False):
    x = np.asarray(x, dtype=np.float32)
    W = np.asarray(W, dtype=np.float32)
    b = np.asarray(b, dtype=np.float32)
    gate_values = np.asarray(gate_values, dtype=np.float32)

    # Replicated, layout-prepped weights (hi/lo split for exact logits)
    Wh = _round_mant(W)
    Wl32 = W - Wh
    wTh_np = np.ascontiguousarray(Wh.T)                              # (D, S) f32r
    wTb_np = np.ascontiguousarray(W.T).astype(ml_dtypes.bfloat16)    # (D, S) bf16
    wTl_np = np.ascontiguousarray(Wl32.T).astype(ml_dtypes.bfloat16)
    br_np = np.ascontiguousarray(b.reshape(1, S))                    # (1, S)
    gT_np = np.ascontiguousarray(gate_values.reshape(S, D).T)        # (D, S)

    in_maps = []
    for i in range(B):
        xb = x[i]
        xh = _round_mant(xb)
        xl32 = xb - xh
        in_maps.append({
            "xTh": _tile_major(xh),
            "xTl": _tile_major(xl32).astype(ml_dtypes.bfloat16),
            "xTb": _tile_major(xb).astype(ml_dtypes.bfloat16),
            "xn": np.ascontiguousarray(_p2_np(xb)),
            "wTh": wTh_np,
            "wTb": wTb_np,
            "wTl": wTl_np,
            "br": br_np,
            "gT": gT_np,
        })

    nc = _get_nc()
    res = run_bass_kernel_spmd(nc, in_maps, core_ids=list(range(B)), trace=_trace)

    out = np.empty((B, NN, D), dtype=np.float32)
    counts = np.empty((B, NN), dtype=np.int32)
    for i in range(B):
        out[i] = res.results[i]["outT"].T
        counts[i] = np.rint(res.results[i]["cnt"][0]).astype(np.int32)

    if _trace:
        return (out, counts), res
    return out, counts


# revision 12
# speedup vs baseline: 1.7296x; 1.0218x over previous
"""Trainium2 Bass kernel for nn_NodeRouter (MoE routing) — v6.

v6: v5 + bias row DMA'd first (it gates every tile's PSUM group),
W chunks interleaved with the first x tiles, and phase 2 run
chunk-outer with all 16 xn tiles resident (prefetched during phase 1)
so each chunk's gate-multiply epilogue overlaps the next chunk's
matmuls.

v5: v4 + DMA pipelining (x tile prefetch ahead of W, gate DMAs moved
into phase 2) and counts partition-reduce on GpSimd (no counts matmul,
no PSUM bank swap barrier between phases).

v4: phase-1 logits computed with a 3-term compensated split so top-2
selection is fp32-exact at ~3x the f32r matmul rate:
    v = xh*Wh (f32r) + xl*W (bf16) + x*Wl (bf16),  error ~ xl*Wl ~ 2^-24
where xh = round_m(x), xl = bf16(x - xh), Wh = round_m(W), Wl = bf16(W - Wh).

v3 changes over v2 (both-matmuls f32r):
  - xT shipped in tile-major layout (xTt): one contiguous 512KB DMA per
    l-tile, so phase 1 starts after ~2.5MB instead of ~12MB.
  - bias folded into the logits matmul as a K=1 accumulation step
    (kills the DVE bias-add pass; v stays in PSUM, max/stt read PSUM).
  - count accumulation moved to GpSimd, reading C (SBUF) via (C != 0).
  - phase-2 dtype selectable (f32r or bf16) via NODEROUTER_P2_DT.
"""

import os
import sys
sys.path.insert(0, "/opt/trn_rl_repo")

import ml_dtypes
import numpy as np

import concourse.bacc as bacc
import concourse.bass_isa as bass_isa
import concourse.bass as bass
import concourse.mybir as mybir
import concourse.tile as tile
from concourse.bass_utils import run_bass_kernel_spmd

B, L, D = 8, 2048, 1024
NN, NH, K = 64, 8, 2
S = NN * NH          # 512 gate slots
NT = L // 128        # 16 token tiles
DC = D // 128        # 8 contraction chunks
F32 = mybir.dt.float32

MM_DT = {
    "f32": mybir.dt.float32,
    "f32r": mybir.dt.float32r,
}[os.environ.get("NODEROUTER_MM_DT", "f32r")]
P2_DT = {
    "f32": mybir.dt.float32,
    "f32r": mybir.dt.float32r,
    "bf16": mybir.dt.bfloat16,
}[os.environ.get("NODEROUTER_P2_DT", "f32r")]


def build_nc():
    nc = bacc.Bacc(
        "TRN2",
        target_bir_lowering=False,
        debug=False,
        num_devices=B,
    )

    BF16 = mybir.dt.bfloat16
    F32R = mybir.dt.float32r
    # tile-major x^T: row block i holds (128 d, 8*128) where cols [c*128, ...)
    # are chunk c of l-tile i, partitions = d within chunk, cols = token l.
    xTh = nc.dram_tensor("xTh", [NT * 128, DC * 128], F32R, kind="ExternalInput").ap()
    xTl = nc.dram_tensor("xTl", [NT * 128, DC * 128], BF16, kind="ExternalInput").ap()
    xTb = nc.dram_tensor("xTb", [NT * 128, DC * 128], BF16, kind="ExternalInput").ap()
    xn = nc.dram_tensor("xn", [L, D], P2_DT, kind="ExternalInput").ap()
    wTh = nc.dram_tensor("wTh", [D, S], F32R, kind="ExternalInput").ap()
    wTb = nc.dram_tensor("wTb", [D, S], BF16, kind="ExternalInput").ap()
    wTl = nc.dram_tensor("wTl", [D, S], BF16, kind="ExternalInput").ap()
    br = nc.dram_tensor("br", [1, S], F32R, kind="ExternalInput").ap()
    gT = nc.dram_tensor("gT", [D, S], F32, kind="ExternalInput").ap()
    outT = nc.dram_tensor("outT", [D, NN], F32, kind="ExternalOutput").ap()
    cnt = nc.dram_tensor("cnt", [1, NN], F32, kind="ExternalOutput").ap()

    ts = bass.ts
    ige = mybir.AluOpType.is_ge
    ine = mybir.AluOpType.not_equal
    mul = mybir.AluOpType.mult
    add = mybir.AluOpType.add

    with tile.TileContext(nc) as tc, tc.tile_pool(name="const", bufs=1) as const:

        # Resident SBUF tensors (DMAs for x tiles 0-2 are issued first so
        # phase 1 starts as soon as ~1.5MB lands; W streams in behind them)
        wth_sb = const.tile([128, DC * S], F32R)      # W^T hi, chunk c at cols [c*S, ...)
        wtb_sb = const.tile([128, DC * S], BF16)      # W^T bf16
        wtl_sb = const.tile([128, DC * S], BF16)      # W^T lo residual
        br_sb = const.tile([1, S], F32R)
        gt_sb = const.tile([128, DC * S], F32)
        c_all = const.tile([128, NT * S], P2_DT)      # routing matrix, tile i at cols [i*S, ...)
        macc = const.tile([128, S], F32)              # mask accumulator
        nc.vector.memset(macc[:], 0.0)
        ones1p = const.tile([1, 128], F32R)           # K=1 bias-matmul lhsT
        cnt_sb = const.tile([1, NN], F32)
        outT_sb = const.tile([128, DC * NN], F32)
        xn_all = const.tile([128, NT * D], P2_DT)     # xn tile i at cols [i*D, ...)

        # ---- Phase 1: logits (+bias), top-2 threshold, routing matrix C ----
        PREF = 3
        with tc.tile_pool(name="vps", bufs=2, space="PSUM") as vps_pool, \
             tc.tile_pool(name="xtt", bufs=PREF) as xtt_pool, \
             tc.tile_pool(name="work", bufs=2) as work_pool, \
             tc.tile_pool(name="mx", bufs=2) as mx_pool:
            xh_t, xl_t, xb_t = {}, {}, {}

            def issue_x(j):
                xh_t[j] = xtt_pool.tile([128, DC * 128], F32R, name=f"xh{j}", tag="xh")
                nc.sync.dma_start(xh_t[j][:], xTh[j * 128:(j + 1) * 128, :])
                xl_t[j] = xtt_pool.tile([128, DC * 128], BF16, name=f"xl{j}", tag="xl")
                nc.sync.dma_start(xl_t[j][:], xTl[j * 128:(j + 1) * 128, :])
                xb_t[j] = xtt_pool.tile([128, DC * 128], BF16, name=f"xb{j}", tag="xb")
                nc.sync.dma_start(xb_t[j][:], xTb[j * 128:(j + 1) * 128, :])

            # bias row first: it gates every tile's PSUM accumulation group
            nc.sync.dma_start(br_sb[:], br[:])
            nc.vector.tensor_scalar(
                ones1p[:], br_sb[:, 0:128].bitcast(F32), 0.0, 1.0, op0=mul, op1=add
            )

            def issue_w(c):
                nc.sync.dma_start(wth_sb[:, ts(c, S)], wTh[c * 128:(c + 1) * 128, :])
                nc.sync.dma_start(wtb_sb[:, ts(c, S)], wTb[c * 128:(c + 1) * 128, :])
                nc.sync.dma_start(wtl_sb[:, ts(c, S)], wTl[c * 128:(c + 1) * 128, :])

            # interleave W chunks with the first x tiles so tile 0 can start
            # after ~1.5MB: Wc0, x0, Wc1, Wc2, x1, Wc3-4, x2, Wc5-7
            issue_w(0)
            issue_x(0)
            issue_w(1); issue_w(2)
            issue_x(1)
            issue_w(3); issue_w(4)
            issue_x(2)
            issue_w(5); issue_w(6); issue_w(7)

            for i in range(NT):
                if i + PREF < NT:
                    issue_x(i + PREF)
                # pace the phase-2 operand loads through phase 1 (~0.6MB/tile
                # of spare DMA bandwidth): xn tile i, and gate chunks late
                nc.sync.dma_start(
                    xn_all[:, i * D:(i + 1) * D], xn[i * 128:(i + 1) * 128, :]
                )
                if i >= NT - DC:
                    c = i - (NT - DC)
                    nc.sync.dma_start(
                        gt_sb[:, ts(c, S)], gT[c * 128:(c + 1) * 128, :]
                    )
                xh_sb, xl_sb, xb_sb = xh_t.pop(i), xl_t.pop(i), xb_t.pop(i)
                v_ps = vps_pool.tile([128, S], F32)
                # bias via K=1 broadcast matmul, then 3 compensated terms
                # per contraction chunk: xh*Wh (f32r) + xl*W (bf16) + x*Wl (bf16)
                nc.tensor.matmul(
                    v_ps[:], lhsT=ones1p[:], rhs=br_sb[:],
                    start=True, stop=False,
                )
                for c in range(DC):
                    nc.tensor.matmul(
                        v_ps[:], lhsT=xh_sb[:, ts(c, 128)], rhs=wth_sb[:, ts(c, S)],
                        start=False, stop=False,
                    )
                    nc.tensor.matmul(
                        v_ps[:], lhsT=xl_sb[:, ts(c, 128)], rhs=wtb_sb[:, ts(c, S)],
                        start=False, stop=False,
                    )
                    nc.tensor.matmul(
                        v_ps[:], lhsT=xb_sb[:, ts(c, 128)], rhs=wtl_sb[:, ts(c, S)],
                        start=False, stop=(c == DC - 1),
                    )
                # ACT engine moves v to SBUF (frees DVE from the copy)
                vb = work_pool.tile([128, S], F32)
                nc.scalar.copy(vb[:], v_ps[:])
                mx = mx_pool.tile([128, 8], F32)
                nc.vector.max(mx[:], vb[:])
                t_ap = mx[:, K - 1:K]  # 2nd-largest per token
                # C = (v >= t) * v
                nc.vector.scalar_tensor_tensor(
                    c_all[:, ts(i, S)], vb[:], t_ap, vb[:], op0=ige, op1=mul
                )
                # macc += (v >= t)
                nc.vector.scalar_tensor_tensor(
                    macc[:], vb[:], t_ap, macc[:],
                    op0=ige, op1=add,
                )

            # counts: reduce heads on free dim (DVE), then partitions (GpSimd)
            cnt_h = work_pool.tile([128, NN], F32, name="cnt_h", tag="cnt_h", bufs=1)
            nc.vector.tensor_reduce(
                cnt_h[:],
                macc[:].rearrange("p (n h) -> p n h", h=NH),
                axis=mybir.AxisListType.X,
                op=add,
            )
            cnt_r = work_pool.tile([128, NN], F32, name="cnt_r", tag="cnt_r", bufs=1)
            nc.gpsimd.partition_all_reduce(
                cnt_r[:], cnt_h[:], channels=128, reduce_op=bass_isa.ReduceOp.add
            )
            nc.vector.tensor_copy(cnt_sb[:], cnt_r[0:1, :])

        # ---- Phase 2: A^T[d, s] = sum_l x[l, d] * C[l, s] ----
        # chunk-outer: chunk c's epilogue overlaps chunk c+1's matmuls.
        # All 16 xn tiles live in SBUF, prefetched during phase 1.
        with tc.tile_pool(name="atps", bufs=2, space="PSUM") as at_pool, \
             tc.tile_pool(name="gw", bufs=2) as gw_pool:
            for c in range(DC):
                at_ps = at_pool.tile([128, S], F32, name=f"at{c}", tag="at")
                for i in range(NT):
                    nc.tensor.matmul(
                        at_ps[:],
                        lhsT=xn_all[:, i * D + c * 128: i * D + (c + 1) * 128],
                        rhs=c_all[:, ts(i, S)],
                        start=(i == 0),
                        stop=(i == NT - 1),
                    )
                gtile = gw_pool.tile([128, S], F32)
                nc.vector.tensor_mul(gtile[:], at_ps[:], gt_sb[:, ts(c, S)])
                nc.vector.tensor_reduce(
                    outT_sb[:, ts(c, NN)],
                    gtile[:].rearrange("p (n h) -> p n h", h=NH),
                    axis=mybir.AxisListType.X,
                    op=add,
                )
                nc.sync.dma_start(
                    outT[c * 128:(c + 1) * 128, :], outT_sb[:, ts(c, NN)]
                )
            nc.sync.dma_start(cnt[:], cnt_sb[:])

    nc.compile()
    return nc


_NC_CACHE = None


def _get_nc():
    global _NC_CACHE
    if _NC_CACHE is None:
        _NC_CACHE = build_nc()
    return _NC_CACHE


def _p2_np(a):
    if P2_DT == mybir.dt.bfloat16:
        return a.astype(ml_dtypes.bfloat16)
    return a


# f32r operand mantissa width (explicit bits) used for the hi/lo split.
SPLIT_MANT = int(os.environ.get("NODEROUTER_SPLIT_MANT", "11"))


def _round_mant(a, mant=SPLIT_MANT):
    """Round-to-nearest keeping `mant` explicit mantissa bits (f32-exact)."""
    am = np.abs(a)
    e = np.floor(np.log2(am, where=am > 0, out=np.zeros_like(a)))
    scale = np.float32(2.0) ** (e - mant).astype(np.float32)
    return (np.round(a / scale) * scale).astype(np.float32)


def _tile_major(xb):
    """[i*128+l, c*128+p] -> [i, p, c, l] tile-major transpose, kept 2D."""
    return np.ascontiguousarray(
        xb.reshape(NT, 128, DC, 128).transpose(0, 3, 2, 1).reshape(NT * 128, D)
    )


def kernel(x, W, b, gate_values, _trace=False):
    x = np.asarray(x, dtype=np.float32)
    W = np.asarray(W, dtype=np.float32)
    b = np.asarray(b, dtype=np.float32)
    gate_values = np.asarray(gate_values, dtype=np.float32)

    # Replicated, layout-prepped weights (hi/lo split for exact logits)
    Wh = _round_mant(W)
    Wl32 = W - Wh
    wTh_np = np.ascontiguousarray(Wh.T)                              # (D, S) f32r
    wTb_np = np.ascontiguousarray(W.T).astype(ml_dtypes.bfloat16)    # (D, S) bf16
    wTl_np = np.ascontiguousarray(Wl32.T).astype(ml_dtypes.bfloat16)
    br_np = np.ascontiguousarray(b.reshape(1, S))                    # (1, S)
    gT_np = np.ascontiguousarray(gate_values.reshape(S, D).T)        # (D, S)

    in_maps = []
    for i in range(B):
        xb = x[i]
        xh = _round_mant(xb)
        xl32 = xb - xh
        in_maps.append({
            "xTh": _tile_major(xh),
            "xTl": _tile_major(xl32).astype(ml_dtypes.bfloat16),
            "xTb": _tile_major(xb).astype(ml_dtypes.bfloat16),
            "xn": np.ascontiguousarray(_p2_np(xb)),
            "wTh": wTh_np,
            "wTb": wTb_np,
            "wTl": wTl_np,
            "br": br_np,
            "gT": gT_np,
        })

    nc = _get_nc()
    res = run_bass_kernel_spmd(nc, in_maps, core_ids=list(range(B)), trace=_trace)

    out = np.empty((B, NN, D), dtype=np.float32)
    counts = np.empty((B, NN), dtype=np.int32)
    for i in range(B):
        out[i] = res.results[i]["outT"].T
        counts[i] = np.rint(res.results[i]["cnt"][0]).astype(np.int32)

    if _trace:
        return (out, counts), res
    return out, counts
